# revision 7
# baseline (speedup 1.0000x reference)
"""Llama4 MoE experts + shared LoRA, expert-parallel on 8 TRN2 NeuronCores.

Per-core (expert e): x[1024,1024] @ W_gu[1024,4096] (+ rank-8 LoRA) -> SwiGLU
-> h[1024,2048] @ W_dn[2048,1024] (+ rank-8 LoRA) -> out[1024,1024].

Layout strategy: all contractions keep the reduced dim on partitions and the
intermediate transposed. x arrives from the host already transposed ([H, T]),
as do the tiny LoRA factors, so the kernel runs zero on-chip transposes:
gate_up^T tiles [128 F-part, 512 T-free] come out of PSUM with H contracted,
SwiGLU produces hiddenT with I on partitions -- the contraction layout the
down matmul needs -- and the down output lands as natural [T, H] rows.

All matmuls run in float32r (1 cycle/row for free-dim >= 256). Weight DMAs
are shaped for >=1KiB contiguous runs. The down-proj weight stream and the
output writes issue on the GpSimd DMA queue so they never queue behind the
gate_up weight stream on the Sync queue.
"""
import sys

sys.path.insert(0, "/opt/trn_rl_repo")

import numpy as np

import concourse.bacc as bacc
import concourse.bass as bass
import concourse.mybir as mybir
import concourse.tile as tile
from concourse.bass_utils import run_bass_kernel_spmd

E = 8           # experts == cores
T = 1024        # tokens per expert
H = 1024        # hidden
I = 2048        # intermediate
F2 = 2 * I      # gate+up
R = 8           # lora rank
SCALING = 2.0   # lora_alpha / rank
P = 128         # partitions
NFREE = 512     # moving free-dim per matmul (one PSUM bank of fp32)
KH = H // P     # 8 k-tiles over H
KI = I // P     # 16 k-tiles over I
NT = T // NFREE     # 2 T-chunks
FC = 256            # gate-column chunk per weight DMA
NFC = I // FC       # 8 weight-chunk iterations
JW = FC // P        # 2 f-subtiles per chunk
KO_DN = 4           # down-proj k-tiles packed per DMA

F32 = mybir.dt.float32
F32R = mybir.dt.float32r


def build_kernel():
    nc = bacc.Bacc("TRN2", target_bir_lowering=False, debug=False)

    xT_d = nc.dram_tensor("xT", [H, T], F32R, kind="ExternalInput")
    wgu_d = nc.dram_tensor("w_gu", [H, F2], F32R, kind="ExternalInput")
    wdn_d = nc.dram_tensor("w_dn", [I, H], F32R, kind="ExternalInput")
    aguT_d = nc.dram_tensor("a_guT", [H, R], F32R, kind="ExternalInput")
    bguT_d = nc.dram_tensor("b_guT", [R, F2], F32R, kind="ExternalInput")
    adnT_d = nc.dram_tensor("a_dnT", [I, R], F32R, kind="ExternalInput")
    bdnT_d = nc.dram_tensor("b_dnT", [R, H], F32R, kind="ExternalInput")
    out_d = nc.dram_tensor("out", [T, H], F32, kind="ExternalOutput")

    with tile.TileContext(nc) as tc:
        with (
            tc.tile_pool(name="xT", bufs=1) as xT_pool,
            tc.tile_pool(name="hT", bufs=1) as hT_pool,
            tc.tile_pool(name="smalls", bufs=1) as small_pool,
            tc.tile_pool(name="wg", bufs=2) as wg_pool,
            tc.tile_pool(name="wu", bufs=2) as wu_pool,
            tc.tile_pool(name="wdn", bufs=2) as wdn_pool,
            tc.tile_pool(name="silu", bufs=3) as silu_pool,
            tc.tile_pool(name="outs", bufs=4) as out_pool,
            tc.tile_pool(name="ps", bufs=8, space="PSUM") as ps_pool,
        ):
            # ---- resident inputs, all pre-transposed on the host ----
            # DMA issue order is the latency-critical path at startup:
            # x t-half 0 + A_gu + wg/wu chunk 0 unblock the first compute.
            xt = xT_pool.tile([P, KH, T], F32R, tag="xT")
            nc.sync.dma_start(
                xt[:, :, 0:NFREE],
                xT_d[:, 0:NFREE].rearrange("(ko ki) t -> ki ko t", ki=P))
            aguT = small_pool.tile([P, KH, R], F32R, tag="aguT")
            nc.sync.dma_start(aguT[:],
                              aguT_d[:].rearrange("(ko ki) r -> ki ko r", ki=P))

            wgs, wus = [], []
            def load_wgu(fc):
                wg = wg_pool.tile([P, KH, FC], F32R, tag="wg")
                wu = wu_pool.tile([P, KH, FC], F32R, tag="wu")
                fg, fu = FC * fc, I + FC * fc
                nc.sync.dma_start(
                    wg[:], wgu_d[:, fg:fg + FC].rearrange(
                        "(ko ki) f -> ki ko f", ki=P))
                nc.sync.dma_start(
                    wu[:], wgu_d[:, fu:fu + FC].rearrange(
                        "(ko ki) f -> ki ko f", ki=P))
                wgs.append(wg)
                wus.append(wu)

            load_wgu(0)
            bguT = small_pool.tile([R, F2], F32R, tag="bguT")
            nc.sync.dma_start(bguT[:], bguT_d[:])
            nc.sync.dma_start(
                xt[:, :, NFREE:T],
                xT_d[:, NFREE:T].rearrange("(ko ki) t -> ki ko t", ki=P))
            adnT = small_pool.tile([P, KI, R], F32R, tag="adnT")
            nc.sync.dma_start(adnT[:],
                              adnT_d[:].rearrange("(ko ki) r -> ki ko r", ki=P))
            bdnT = small_pool.tile([R, H], F32R, tag="bdnT")
            nc.sync.dma_start(bdnT[:], bdnT_d[:])

            # ---- r1T = SCALING * (A_gu @ x^T): [8 R, 1024 T] ----
            r1T = small_pool.tile([R, T], F32R, tag="r1T")
            def r1_chunk(t):
                ts = slice(NFREE * t, NFREE * (t + 1))
                ps = ps_pool.tile([R, NFREE], F32, tag="ps")
                for k in range(KH):
                    nc.tensor.matmul(ps[:], aguT[:, k, :], xt[:, k, ts],
                                     start=(k == 0), stop=(k == KH - 1))
                nc.vector.tensor_scalar_mul(r1T[:, ts], ps[:], SCALING)

            # ---- gate_up^T + SwiGLU -> hiddenT[i] [128 I, 1024 T] ----
            hT = [hT_pool.tile([P, T], F32R, tag=f"hT{k}", name=f"hT{k}")
                  for k in range(KI)]

            def gu_unit(fc, j, t, interleave=True):
                fg, fu = FC * fc, I + FC * fc
                i = JW * fc + j
                fs = slice(P * j, P * (j + 1))
                ts = slice(NFREE * t, NFREE * (t + 1))
                wg, wu = wgs[fc], wus[fc]
                psg = ps_pool.tile([P, NFREE], F32, tag="ps")
                psu = ps_pool.tile([P, NFREE], F32, tag="ps")
                # interleave the two accumulation chains so chain boundaries
                # hide in each other's moving passes (sequential at startup,
                # where the g chain must not wait for the wu DMA)
                if interleave:
                    for k in range(KH):
                        nc.tensor.matmul(psg[:], wg[:, k, fs], xt[:, k, ts],
                                         start=(k == 0), stop=False)
                        nc.tensor.matmul(psu[:], wu[:, k, fs], xt[:, k, ts],
                                         start=(k == 0), stop=False)
                else:
                    for k in range(KH):
                        nc.tensor.matmul(psg[:], wg[:, k, fs], xt[:, k, ts],
                                         start=(k == 0), stop=False)
                    for k in range(KH):
                        nc.tensor.matmul(psu[:], wu[:, k, fs], xt[:, k, ts],
                                         start=(k == 0), stop=False)
                nc.tensor.matmul(psg[:], bguT[:, fg + P * j:fg + P * (j + 1)],
                                 r1T[:, ts], start=False, stop=True)
                nc.tensor.matmul(psu[:], bguT[:, fu + P * j:fu + P * (j + 1)],
                                 r1T[:, ts], start=False, stop=True)
                sg = silu_pool.tile([P, NFREE], F32, tag="silu")
                nc.scalar.activation(sg[:], psg[:],
                                     mybir.ActivationFunctionType.Silu)
                nc.vector.tensor_mul(hT[i][:, ts], sg[:], psu[:])

            # fc0 runs t-major so compute starts before x t-half 1 lands
            r1_chunk(0)
            gu_unit(0, 0, 0, interleave=False)
            gu_unit(0, 1, 0, interleave=False)
            r1_chunk(1)
            gu_unit(0, 0, 1)
            gu_unit(0, 1, 1)
            for fc in range(1, NFC):
                load_wgu(fc)
                for j in range(JW):
                    for t in range(NT):
                        gu_unit(fc, j, t)

            # ---- r2T = SCALING * (A_dn @ hidden^T): [8 R, 1024 T] ----
            r2T = small_pool.tile([R, T], F32R, tag="r2T")
            for t in range(NT):
                ts = slice(NFREE * t, NFREE * (t + 1))
                ps = ps_pool.tile([R, NFREE], F32, tag="ps")
                for k in range(KI):
                    nc.tensor.matmul(ps[:], adnT[:, k, :], hT[k][:, ts],
                                     start=(k == 0), stop=(k == KI - 1))
                nc.vector.tensor_scalar_mul(r2T[:, ts], ps[:], SCALING)

            # ---- out[T, H] = hidden @ W_dn + lora ----
            for hg in range(NT):
                hs = slice(NFREE * hg, NFREE * (hg + 1))
                pos = [ps_pool.tile([P, NFREE], F32, tag="ps",
                                    name=f"po{hg}_{j}") for j in range(T // P)]
                for kg in range(KI // KO_DN):
                    wd = wdn_pool.tile([P, KO_DN, NFREE], F32R, tag="wdn")
                    r0 = P * KO_DN * kg
                    nc.sync.dma_start(
                        wd[:], wdn_d[r0:r0 + P * KO_DN, hs].rearrange(
                            "(ko ki) h -> ki ko h", ki=P))
                    for ko in range(KO_DN):
                        k = KO_DN * kg + ko
                        for j in range(T // P):
                            nc.tensor.matmul(pos[j][:],
                                             hT[k][:, P * j:P * (j + 1)],
                                             wd[:, ko, :],
                                             start=(k == 0), stop=False)
                for j in range(T // P):
                    nc.tensor.matmul(pos[j][:], r2T[:, P * j:P * (j + 1)],
                                     bdnT[:, hs], start=False, stop=True)
                    ot = out_pool.tile([P, NFREE], F32, tag="outs")
                    # alternate copy engines and DMA queues so the final
                    # drain runs two-wide
                    if j % 2 == 0:
                        nc.scalar.activation(ot[:], pos[j][:],
                                             mybir.ActivationFunctionType.Copy)
                        nc.sync.dma_start(out_d[P * j:P * (j + 1), hs], ot[:])
                    else:
                        nc.vector.tensor_copy(ot[:], pos[j][:])
                        nc.scalar.dma_start(out_d[P * j:P * (j + 1), hs], ot[:])

    nc.finalize()
    return nc


_NC_CACHE = None


def _get_nc():
    global _NC_CACHE
    if _NC_CACHE is None:
        _NC_CACHE = build_kernel()
    return _NC_CACHE


def _run(hidden_states, gate_up_proj, down_proj,
         lora_A_gu, lora_B_gu, lora_A_dn, lora_B_dn, **spmd_kwargs):
    f32 = np.float32
    hidden_states = np.asarray(hidden_states, dtype=f32)
    gate_up_proj = np.ascontiguousarray(gate_up_proj, dtype=f32)
    down_proj = np.ascontiguousarray(down_proj, dtype=f32)
    aguT = np.ascontiguousarray(np.asarray(lora_A_gu, dtype=f32).T)
    bguT = np.ascontiguousarray(np.asarray(lora_B_gu, dtype=f32).T)
    adnT = np.ascontiguousarray(np.asarray(lora_A_dn, dtype=f32).T)
    bdnT = np.ascontiguousarray(np.asarray(lora_B_dn, dtype=f32).T)

    nc = _get_nc()
    in_maps = []
    for e in range(E):
        in_maps.append({
            "xT": np.ascontiguousarray(hidden_states[T * e:T * (e + 1), :].T),
            "w_gu": gate_up_proj[e],
            "w_dn": down_proj[e],
            "a_guT": aguT,
            "b_guT": bguT,
            "a_dnT": adnT,
            "b_dnT": bdnT,
        })
    res = run_bass_kernel_spmd(nc, in_maps, core_ids=list(range(E)),
                               **spmd_kwargs)
    out = np.concatenate([res.results[e]["out"] for e in range(E)], axis=0)
    return out, res


def kernel(hidden_states, gate_up_proj, down_proj,
           lora_A_gu, lora_B_gu, lora_A_dn, lora_B_dn):
    out, _ = _run(hidden_states, gate_up_proj, down_proj,
                  lora_A_gu, lora_B_gu, lora_A_dn, lora_B_dn)
    return out


# revision 10
# speedup vs baseline: 1.0059x; 1.0059x over previous
"""Llama4 MoE experts + shared LoRA, expert-parallel on 8 TRN2 NeuronCores.

Per-core (expert e): x[1024,1024] @ W_gu[1024,4096] (+ rank-8 LoRA) -> SwiGLU
-> h[1024,2048] @ W_dn[2048,1024] (+ rank-8 LoRA) -> out[1024,1024].

Layout strategy: all contractions keep the reduced dim on partitions and the
intermediate transposed. x arrives from the host already transposed ([H, T]),
as do the tiny LoRA factors, so the kernel runs zero on-chip transposes:
gate_up^T tiles [128 F-part, 512 T-free] come out of PSUM with H contracted,
SwiGLU produces hiddenT with I on partitions -- the contraction layout the
down matmul needs -- and the down output lands as natural [T, H] rows.

All matmuls run in float32r (1 cycle/row for free-dim >= 256). Weight DMAs
are shaped for >=1KiB contiguous runs. The down-proj weight stream and the
output writes issue on the GpSimd DMA queue so they never queue behind the
gate_up weight stream on the Sync queue.
"""
import sys

sys.path.insert(0, "/opt/trn_rl_repo")

import numpy as np

import concourse.bacc as bacc
import concourse.bass as bass
import concourse.mybir as mybir
import concourse.tile as tile
from concourse.bass_utils import run_bass_kernel_spmd

E = 8           # experts == cores
T = 1024        # tokens per expert
H = 1024        # hidden
I = 2048        # intermediate
F2 = 2 * I      # gate+up
R = 8           # lora rank
SCALING = 2.0   # lora_alpha / rank
P = 128         # partitions
NFREE = 512     # moving free-dim per matmul (one PSUM bank of fp32)
KH = H // P     # 8 k-tiles over H
KI = I // P     # 16 k-tiles over I
NT = T // NFREE     # 2 T-chunks
FC = 256            # gate-column chunk per weight DMA
NFC = I // FC       # 8 weight-chunk iterations
JW = FC // P        # 2 f-subtiles per chunk
KO_DN = 4           # down-proj k-tiles packed per DMA

F32 = mybir.dt.float32
F32R = mybir.dt.float32r


def build_kernel():
    nc = bacc.Bacc("TRN2", target_bir_lowering=False, debug=False)

    xT_d = nc.dram_tensor("xT", [H, T], F32R, kind="ExternalInput")
    wgu_d = nc.dram_tensor("w_gu", [H, F2], F32R, kind="ExternalInput")
    wdn_d = nc.dram_tensor("w_dn", [I, H], F32R, kind="ExternalInput")
    aguT_d = nc.dram_tensor("a_guT", [H, R], F32R, kind="ExternalInput")
    bguT_d = nc.dram_tensor("b_guT", [R, F2], F32R, kind="ExternalInput")
    adnT_d = nc.dram_tensor("a_dnT", [I, R], F32R, kind="ExternalInput")
    bdnT_d = nc.dram_tensor("b_dnT", [R, H], F32R, kind="ExternalInput")
    out_d = nc.dram_tensor("out", [T, H], F32, kind="ExternalOutput")

    with tile.TileContext(nc) as tc:
        with (
            tc.tile_pool(name="xT", bufs=1) as xT_pool,
            tc.tile_pool(name="hT", bufs=1) as hT_pool,
            tc.tile_pool(name="smalls", bufs=1) as small_pool,
            tc.tile_pool(name="wg", bufs=2) as wg_pool,
            tc.tile_pool(name="wu", bufs=2) as wu_pool,
            tc.tile_pool(name="wdn", bufs=2) as wdn_pool,
            tc.tile_pool(name="silu", bufs=3) as silu_pool,
            tc.tile_pool(name="outs", bufs=6) as out_pool,
            tc.tile_pool(name="ps", bufs=8, space="PSUM") as ps_pool,
        ):
            # ---- resident inputs, all pre-transposed on the host ----
            # DMA issue order is the latency-critical path at startup:
            # x t-half 0 + A_gu + wg/wu chunk 0 unblock the first compute.
            xt = xT_pool.tile([P, KH, T], F32R, tag="xT")
            nc.sync.dma_start(
                xt[:, :, 0:NFREE],
                xT_d[:, 0:NFREE].rearrange("(ko ki) t -> ki ko t", ki=P))
            aguT = small_pool.tile([P, KH, R], F32R, tag="aguT")
            nc.sync.dma_start(aguT[:],
                              aguT_d[:].rearrange("(ko ki) r -> ki ko r", ki=P))

            wgs, wus = [], []
            def load_wgu(fc):
                wg = wg_pool.tile([P, KH, FC], F32R, tag="wg")
                wu = wu_pool.tile([P, KH, FC], F32R, tag="wu")
                fg, fu = FC * fc, I + FC * fc
                nc.sync.dma_start(
                    wg[:], wgu_d[:, fg:fg + FC].rearrange(
                        "(ko ki) f -> ki ko f", ki=P))
                nc.sync.dma_start(
                    wu[:], wgu_d[:, fu:fu + FC].rearrange(
                        "(ko ki) f -> ki ko f", ki=P))
                wgs.append(wg)
                wus.append(wu)

            load_wgu(0)
            bguT = small_pool.tile([R, F2], F32R, tag="bguT")
            nc.sync.dma_start(bguT[:], bguT_d[:])
            nc.sync.dma_start(
                xt[:, :, NFREE:T],
                xT_d[:, NFREE:T].rearrange("(ko ki) t -> ki ko t", ki=P))
            adnT = small_pool.tile([P, KI, R], F32R, tag="adnT")
            nc.sync.dma_start(adnT[:],
                              adnT_d[:].rearrange("(ko ki) r -> ki ko r", ki=P))
            bdnT = small_pool.tile([R, H], F32R, tag="bdnT")
            nc.sync.dma_start(bdnT[:], bdnT_d[:])

            # ---- r1T = SCALING * (A_gu @ x^T): [8 R, 1024 T] ----
            r1T = small_pool.tile([R, T], F32R, tag="r1T")
            def r1_chunk(t):
                ts = slice(NFREE * t, NFREE * (t + 1))
                ps = ps_pool.tile([R, NFREE], F32, tag="ps")
                for k in range(KH):
                    nc.tensor.matmul(ps[:], aguT[:, k, :], xt[:, k, ts],
                                     start=(k == 0), stop=(k == KH - 1))
                nc.vector.tensor_scalar_mul(r1T[:, ts], ps[:], SCALING)

            # ---- gate_up^T + SwiGLU -> hiddenT[i] [128 I, 1024 T] ----
            hT = [hT_pool.tile([P, T], F32R, tag=f"hT{k}", name=f"hT{k}")
                  for k in range(KI)]

            def gu_unit(fc, j, t, interleave=True):
                fg, fu = FC * fc, I + FC * fc
                i = JW * fc + j
                fs = slice(P * j, P * (j + 1))
                ts = slice(NFREE * t, NFREE * (t + 1))
                wg, wu = wgs[fc], wus[fc]
                psg = ps_pool.tile([P, NFREE], F32, tag="ps")
                psu = ps_pool.tile([P, NFREE], F32, tag="ps")
                # interleave the two accumulation chains so chain boundaries
                # hide in each other's moving passes (sequential at startup,
                # where the g chain must not wait for the wu DMA)
                if interleave:
                    for k in range(KH):
                        nc.tensor.matmul(psg[:], wg[:, k, fs], xt[:, k, ts],
                                         start=(k == 0), stop=False)
                        nc.tensor.matmul(psu[:], wu[:, k, fs], xt[:, k, ts],
                                         start=(k == 0), stop=False)
                else:
                    for k in range(KH):
                        nc.tensor.matmul(psg[:], wg[:, k, fs], xt[:, k, ts],
                                         start=(k == 0), stop=False)
                    for k in range(KH):
                        nc.tensor.matmul(psu[:], wu[:, k, fs], xt[:, k, ts],
                                         start=(k == 0), stop=False)
                nc.tensor.matmul(psg[:], bguT[:, fg + P * j:fg + P * (j + 1)],
                                 r1T[:, ts], start=False, stop=True)
                nc.tensor.matmul(psu[:], bguT[:, fu + P * j:fu + P * (j + 1)],
                                 r1T[:, ts], start=False, stop=True)
                sg = silu_pool.tile([P, NFREE], F32, tag="silu")
                nc.scalar.activation(sg[:], psg[:],
                                     mybir.ActivationFunctionType.Silu)
                nc.vector.tensor_mul(hT[i][:, ts], sg[:], psu[:])

            def gu_pair(fc, t):
                # both j-subtiles of a chunk as 4 interleaved chains --
                # consecutive group starts/stops pipeline instead of each
                # paying the accumulation-group boundary latency
                fg, fu = FC * fc, I + FC * fc
                ts = slice(NFREE * t, NFREE * (t + 1))
                wg, wu = wgs[fc], wus[fc]
                ps = [ps_pool.tile([P, NFREE], F32, tag="ps",
                                   name=f"gu{fc}_{t}_{c}")
                      for c in range(2 * JW)]
                for k in range(KH):
                    for j in range(JW):
                        fs = slice(P * j, P * (j + 1))
                        nc.tensor.matmul(ps[2 * j][:], wg[:, k, fs],
                                         xt[:, k, ts], start=(k == 0),
                                         stop=False)
                        nc.tensor.matmul(ps[2 * j + 1][:], wu[:, k, fs],
                                         xt[:, k, ts], start=(k == 0),
                                         stop=False)
                for j in range(JW):
                    nc.tensor.matmul(ps[2 * j][:],
                                     bguT[:, fg + P * j:fg + P * (j + 1)],
                                     r1T[:, ts], start=False, stop=True)
                    nc.tensor.matmul(ps[2 * j + 1][:],
                                     bguT[:, fu + P * j:fu + P * (j + 1)],
                                     r1T[:, ts], start=False, stop=True)
                for j in range(JW):
                    i = JW * fc + j
                    sg = silu_pool.tile([P, NFREE], F32, tag="silu")
                    nc.scalar.activation(sg[:], ps[2 * j][:],
                                         mybir.ActivationFunctionType.Silu)
                    nc.vector.tensor_mul(hT[i][:, ts], sg[:], ps[2 * j + 1][:])

            # fc0 runs t-major so compute starts before x t-half 1 lands
            r1_chunk(0)
            gu_unit(0, 0, 0, interleave=False)
            gu_unit(0, 1, 0, interleave=False)
            r1_chunk(1)
            gu_pair(0, 1)
            for fc in range(1, NFC):
                load_wgu(fc)
                for t in range(NT):
                    gu_pair(fc, t)

            # ---- r2T = SCALING * (A_dn @ hidden^T): [8 R, 1024 T] ----
            r2T = small_pool.tile([R, T], F32R, tag="r2T")
            for t in range(NT):
                ts = slice(NFREE * t, NFREE * (t + 1))
                ps = ps_pool.tile([R, NFREE], F32, tag="ps")
                for k in range(KI):
                    nc.tensor.matmul(ps[:], adnT[:, k, :], hT[k][:, ts],
                                     start=(k == 0), stop=(k == KI - 1))
                nc.vector.tensor_scalar_mul(r2T[:, ts], ps[:], SCALING)

            # ---- out[T, H] = hidden @ W_dn + lora ----
            for hg in range(NT):
                hs = slice(NFREE * hg, NFREE * (hg + 1))
                pos = [ps_pool.tile([P, NFREE], F32, tag="ps",
                                    name=f"po{hg}_{j}") for j in range(T // P)]
                for kg in range(KI // KO_DN):
                    wd = wdn_pool.tile([P, KO_DN, NFREE], F32R, tag="wdn")
                    r0 = P * KO_DN * kg
                    nc.sync.dma_start(
                        wd[:], wdn_d[r0:r0 + P * KO_DN, hs].rearrange(
                            "(ko ki) h -> ki ko h", ki=P))
                    for ko in range(KO_DN):
                        k = KO_DN * kg + ko
                        for j in range(T // P):
                            nc.tensor.matmul(pos[j][:],
                                             hT[k][:, P * j:P * (j + 1)],
                                             wd[:, ko, :],
                                             start=(k == 0), stop=False)
                for j in range(T // P):
                    nc.tensor.matmul(pos[j][:], r2T[:, P * j:P * (j + 1)],
                                     bdnT[:, hs], start=False, stop=True)
                    ot = out_pool.tile([P, NFREE], F32, tag="outs")
                    # alternate copy engines and DMA queues so the final
                    # drain runs two-wide
                    if j % 2 == 0:
                        nc.scalar.activation(ot[:], pos[j][:],
                                             mybir.ActivationFunctionType.Copy)
                        nc.sync.dma_start(out_d[P * j:P * (j + 1), hs], ot[:])
                    else:
                        nc.vector.tensor_copy(ot[:], pos[j][:])
                        nc.scalar.dma_start(out_d[P * j:P * (j + 1), hs], ot[:])

    nc.finalize()
    return nc


_NC_CACHE = None


def _get_nc():
    global _NC_CACHE
    if _NC_CACHE is None:
        _NC_CACHE = build_kernel()
    return _NC_CACHE


def _run(hidden_states, gate_up_proj, down_proj,
         lora_A_gu, lora_B_gu, lora_A_dn, lora_B_dn, **spmd_kwargs):
    f32 = np.float32
    hidden_states = np.asarray(hidden_states, dtype=f32)
    gate_up_proj = np.ascontiguousarray(gate_up_proj, dtype=f32)
    down_proj = np.ascontiguousarray(down_proj, dtype=f32)
    aguT = np.ascontiguousarray(np.asarray(lora_A_gu, dtype=f32).T)
    bguT = np.ascontiguousarray(np.asarray(lora_B_gu, dtype=f32).T)
    adnT = np.ascontiguousarray(np.asarray(lora_A_dn, dtype=f32).T)
    bdnT = np.ascontiguousarray(np.asarray(lora_B_dn, dtype=f32).T)

    nc = _get_nc()
    in_maps = []
    for e in range(E):
        in_maps.append({
            "xT": np.ascontiguousarray(hidden_states[T * e:T * (e + 1), :].T),
            "w_gu": gate_up_proj[e],
            "w_dn": down_proj[e],
            "a_guT": aguT,
            "b_guT": bguT,
            "a_dnT": adnT,
            "b_dnT": bdnT,
        })
    res = run_bass_kernel_spmd(nc, in_maps, core_ids=list(range(E)),
                               **spmd_kwargs)
    out = np.concatenate([res.results[e]["out"] for e in range(E)], axis=0)
    return out, res


def kernel(hidden_states, gate_up_proj, down_proj,
           lora_A_gu, lora_B_gu, lora_A_dn, lora_B_dn):
    out, _ = _run(hidden_states, gate_up_proj, down_proj,
                  lora_A_gu, lora_B_gu, lora_A_dn, lora_B_dn)
    return out


# revision 13
# speedup vs baseline: 1.1047x; 1.0982x over previous
"""Llama4 MoE experts + shared LoRA, expert-parallel on 8 TRN2 NeuronCores.

Per-core (expert e): x[1024,1024] @ W_gu[1024,4096] (+ rank-8 LoRA) -> SwiGLU
-> h[1024,2048] @ W_dn[2048,1024] (+ rank-8 LoRA) -> out[1024,1024].

Layout strategy: all contractions keep the reduced dim on partitions and the
intermediate transposed. x arrives from the host already transposed ([H, T]),
as do the tiny LoRA factors, so the kernel runs zero on-chip transposes:
gate_up^T tiles [128 F-part, 512 T-free] come out of PSUM with H contracted,
SwiGLU produces hiddenT with I on partitions -- the contraction layout the
down matmul needs -- and the down output lands as natural [T, H] rows.

All matmul operands are bf16 (1 cycle/row, same PE speed as float32r but half
the HBM traffic and SBUF footprint; accumulation stays fp32 in PSUM and the
output is written fp32). Weight DMAs are shaped for >=1KiB contiguous runs.
Only the two hardware DGE queues (sync, scalar) are used -- the gpsimd queue
is a software path that degrades the whole core.
"""
import sys

sys.path.insert(0, "/opt/trn_rl_repo")

import numpy as np
import ml_dtypes

import concourse.bacc as bacc
import concourse.bass as bass
import concourse.mybir as mybir
import concourse.tile as tile
from concourse.bass_utils import run_bass_kernel_spmd

E = 8           # experts == cores
T = 1024        # tokens per expert
H = 1024        # hidden
I = 2048        # intermediate
F2 = 2 * I      # gate+up
R = 8           # lora rank
SCALING = 2.0   # lora_alpha / rank
P = 128         # partitions
NFREE = 512     # moving free-dim per matmul (one PSUM bank of fp32)
KH = H // P     # 8 k-tiles over H
KI = I // P     # 16 k-tiles over I
NT = T // NFREE     # 2 T-chunks
FC = 512            # gate-column chunk per weight DMA
NFC = I // FC       # 4 weight-chunk iterations
JW = FC // P        # 4 f-subtiles per chunk
KO_DN = 4           # down-proj k-tiles packed per DMA

F32 = mybir.dt.float32
F32R = mybir.dt.float32r
BF16 = mybir.dt.bfloat16


def build_kernel():
    nc = bacc.Bacc("TRN2", target_bir_lowering=False, debug=False)

    xT_d = nc.dram_tensor("xT", [H, T], BF16, kind="ExternalInput")
    wgu_d = nc.dram_tensor("w_gu", [H, F2], BF16, kind="ExternalInput")
    wdn_d = nc.dram_tensor("w_dn", [I, H], BF16, kind="ExternalInput")
    aguT_d = nc.dram_tensor("a_guT", [H, R], BF16, kind="ExternalInput")
    bguT_d = nc.dram_tensor("b_guT", [R, F2], F32R, kind="ExternalInput")
    adnT_d = nc.dram_tensor("a_dnT", [I, R], BF16, kind="ExternalInput")
    bdnT_d = nc.dram_tensor("b_dnT", [R, H], F32R, kind="ExternalInput")
    out_d = nc.dram_tensor("out", [T, H], F32, kind="ExternalOutput")

    with tile.TileContext(nc) as tc:
        with (
            tc.tile_pool(name="xT", bufs=1) as xT_pool,
            tc.tile_pool(name="hT", bufs=1) as hT_pool,
            tc.tile_pool(name="smalls", bufs=1) as small_pool,
            tc.tile_pool(name="wg", bufs=2) as wg_pool,
            tc.tile_pool(name="wu", bufs=2) as wu_pool,
            tc.tile_pool(name="wdn", bufs=3) as wdn_pool,
            tc.tile_pool(name="silu", bufs=3) as silu_pool,
            tc.tile_pool(name="outs", bufs=6) as out_pool,
            tc.tile_pool(name="ps", bufs=8, space="PSUM") as ps_pool,
        ):
            # ---- resident inputs, all pre-transposed on the host ----
            # DMA issue order is the latency-critical path at startup:
            # x t-half 0 + A_gu + wg/wu chunk 0 unblock the first compute.
            xt = xT_pool.tile([P, KH, T], BF16, tag="xT")
            nc.sync.dma_start(
                xt[:, :, 0:NFREE],
                xT_d[:, 0:NFREE].rearrange("(ko ki) t -> ki ko t", ki=P))
            aguT = small_pool.tile([P, KH, R], BF16, tag="aguT")
            nc.sync.dma_start(aguT[:],
                              aguT_d[:].rearrange("(ko ki) r -> ki ko r", ki=P))

            wgs, wus = [], []
            def load_wgu(fc):
                wg = wg_pool.tile([P, KH, FC], BF16, tag="wg")
                wu = wu_pool.tile([P, KH, FC], BF16, tag="wu")
                fg, fu = FC * fc, I + FC * fc
                nc.sync.dma_start(
                    wg[:], wgu_d[:, fg:fg + FC].rearrange(
                        "(ko ki) f -> ki ko f", ki=P))
                nc.sync.dma_start(
                    wu[:], wgu_d[:, fu:fu + FC].rearrange(
                        "(ko ki) f -> ki ko f", ki=P))
                wgs.append(wg)
                wus.append(wu)

            load_wgu(0)
            bguT = small_pool.tile([R, F2], F32R, tag="bguT")
            nc.sync.dma_start(bguT[:], bguT_d[:])
            nc.sync.dma_start(
                xt[:, :, NFREE:T],
                xT_d[:, NFREE:T].rearrange("(ko ki) t -> ki ko t", ki=P))
            adnT = small_pool.tile([P, KI, R], BF16, tag="adnT")
            nc.sync.dma_start(adnT[:],
                              adnT_d[:].rearrange("(ko ki) r -> ki ko r", ki=P))
            bdnT = small_pool.tile([R, H], F32R, tag="bdnT")
            nc.sync.dma_start(bdnT[:], bdnT_d[:])

            # ---- r1T = SCALING * (A_gu @ x^T): [8 R, 1024 T] ----
            r1T = small_pool.tile([R, T], F32R, tag="r1T")
            def r1_chunk(t):
                ts = slice(NFREE * t, NFREE * (t + 1))
                ps = ps_pool.tile([R, NFREE], F32, tag="ps")
                for k in range(KH):
                    nc.tensor.matmul(ps[:], aguT[:, k, :], xt[:, k, ts],
                                     start=(k == 0), stop=(k == KH - 1))
                nc.vector.tensor_scalar_mul(r1T[:, ts], ps[:], SCALING)

            # ---- gate_up^T + SwiGLU -> hiddenT[i] [128 I, 1024 T] ----
            hT = [hT_pool.tile([P, T], BF16, tag=f"hT{k}", name=f"hT{k}")
                  for k in range(KI)]

            def gu_pair(fc, t, jh, interleave=True):
                # two j-subtiles as 4 interleaved chains -- consecutive group
                # starts/stops pipeline instead of each paying the boundary
                # latency. interleave=False keeps the g chains first so the
                # startup chunk runs before its wu DMA lands.
                fg, fu = FC * fc, I + FC * fc
                ts = slice(NFREE * t, NFREE * (t + 1))
                wg, wu = wgs[fc], wus[fc]
                js = (2 * jh, 2 * jh + 1)
                ps = [ps_pool.tile([P, NFREE], F32, tag="ps",
                                   name=f"gu{fc}_{t}_{jh}_{c}")
                      for c in range(4)]
                order = range(KH)
                if interleave:
                    for k in order:
                        for c, j in enumerate(js):
                            fs = slice(P * j, P * (j + 1))
                            nc.tensor.matmul(ps[2 * c][:], wg[:, k, fs],
                                             xt[:, k, ts], start=(k == 0),
                                             stop=False)
                            nc.tensor.matmul(ps[2 * c + 1][:], wu[:, k, fs],
                                             xt[:, k, ts], start=(k == 0),
                                             stop=False)
                else:
                    for c, j in enumerate(js):
                        fs = slice(P * j, P * (j + 1))
                        for k in order:
                            nc.tensor.matmul(ps[2 * c][:], wg[:, k, fs],
                                             xt[:, k, ts], start=(k == 0),
                                             stop=False)
                    for c, j in enumerate(js):
                        fs = slice(P * j, P * (j + 1))
                        for k in order:
                            nc.tensor.matmul(ps[2 * c + 1][:], wu[:, k, fs],
                                             xt[:, k, ts], start=(k == 0),
                                             stop=False)
                for c, j in enumerate(js):
                    nc.tensor.matmul(ps[2 * c][:],
                                     bguT[:, fg + P * j:fg + P * (j + 1)],
                                     r1T[:, ts], start=False, stop=True)
                    nc.tensor.matmul(ps[2 * c + 1][:],
                                     bguT[:, fu + P * j:fu + P * (j + 1)],
                                     r1T[:, ts], start=False, stop=True)
                for c, j in enumerate(js):
                    i = JW * fc + j
                    sg = silu_pool.tile([P, NFREE], F32, tag="silu")
                    nc.scalar.activation(sg[:], ps[2 * c][:],
                                         mybir.ActivationFunctionType.Silu)
                    nc.vector.tensor_mul(hT[i][:, ts], sg[:], ps[2 * c + 1][:])

            # fc0 runs t-major so compute starts before x t-half 1 lands
            r1_chunk(0)
            gu_pair(0, 0, 0, interleave=False)
            gu_pair(0, 0, 1)
            r1_chunk(1)
            gu_pair(0, 1, 0)
            gu_pair(0, 1, 1)
            for fc in range(1, NFC):
                load_wgu(fc)
                for t in range(NT):
                    for jh in range(JW // 2):
                        gu_pair(fc, t, jh)

            # ---- r2T = SCALING * (A_dn @ hidden^T): [8 R, 1024 T] ----
            r2T = small_pool.tile([R, T], F32R, tag="r2T")
            for t in range(NT):
                ts = slice(NFREE * t, NFREE * (t + 1))
                ps = ps_pool.tile([R, NFREE], F32, tag="ps")
                for k in range(KI):
                    nc.tensor.matmul(ps[:], adnT[:, k, :], hT[k][:, ts],
                                     start=(k == 0), stop=(k == KI - 1))
                nc.vector.tensor_scalar_mul(r2T[:, ts], ps[:], SCALING)

            # ---- out[T, H] = hidden @ W_dn + lora ----
            for hg in range(NT):
                hs = slice(NFREE * hg, NFREE * (hg + 1))
                pos = [ps_pool.tile([P, NFREE], F32, tag="ps",
                                    name=f"po{hg}_{j}") for j in range(T // P)]
                for kg in range(KI // KO_DN):
                    wd = wdn_pool.tile([P, KO_DN, NFREE], BF16, tag="wdn")
                    r0 = P * KO_DN * kg
                    nc.sync.dma_start(
                        wd[:], wdn_d[r0:r0 + P * KO_DN, hs].rearrange(
                            "(ko ki) h -> ki ko h", ki=P))
                    for ko in range(KO_DN):
                        k = KO_DN * kg + ko
                        for j in range(T // P):
                            nc.tensor.matmul(pos[j][:],
                                             hT[k][:, P * j:P * (j + 1)],
                                             wd[:, ko, :],
                                             start=(k == 0), stop=False)
                for j in range(T // P):
                    nc.tensor.matmul(pos[j][:], r2T[:, P * j:P * (j + 1)],
                                     bdnT[:, hs], start=False, stop=True)
                    ot = out_pool.tile([P, NFREE], F32, tag="outs")
                    # alternate copy engines and DMA queues so the final
                    # drain runs two-wide
                    if j % 2 == 0:
                        nc.scalar.activation(ot[:], pos[j][:],
                                             mybir.ActivationFunctionType.Copy)
                        nc.sync.dma_start(out_d[P * j:P * (j + 1), hs], ot[:])
                    else:
                        nc.vector.tensor_copy(ot[:], pos[j][:])
                        nc.scalar.dma_start(out_d[P * j:P * (j + 1), hs], ot[:])

    nc.finalize()
    return nc


_NC_CACHE = None


def _get_nc():
    global _NC_CACHE
    if _NC_CACHE is None:
        _NC_CACHE = build_kernel()
    return _NC_CACHE


def _run(hidden_states, gate_up_proj, down_proj,
         lora_A_gu, lora_B_gu, lora_A_dn, lora_B_dn, **spmd_kwargs):
    bf16 = ml_dtypes.bfloat16
    xT = np.asarray(hidden_states).T.astype(bf16)      # [H, E*T] view->cast
    wgu = np.asarray(gate_up_proj).astype(bf16)
    wdn = np.asarray(down_proj).astype(bf16)
    aguT = np.ascontiguousarray(np.asarray(lora_A_gu).T.astype(bf16))
    bguT = np.ascontiguousarray(np.asarray(lora_B_gu, dtype=np.float32).T)
    adnT = np.ascontiguousarray(np.asarray(lora_A_dn).T.astype(bf16))
    bdnT = np.ascontiguousarray(np.asarray(lora_B_dn, dtype=np.float32).T)

    nc = _get_nc()
    in_maps = []
    for e in range(E):
        in_maps.append({
            "xT": np.ascontiguousarray(xT[:, T * e:T * (e + 1)]),
            "w_gu": wgu[e],
            "w_dn": wdn[e],
            "a_guT": aguT,
            "b_guT": bguT,
            "a_dnT": adnT,
            "b_dnT": bdnT,
        })
    res = run_bass_kernel_spmd(nc, in_maps, core_ids=list(range(E)),
                               **spmd_kwargs)
    out = np.concatenate([res.results[e]["out"] for e in range(E)], axis=0)
    return out.astype(np.float32), res


def kernel(hidden_states, gate_up_proj, down_proj,
           lora_A_gu, lora_B_gu, lora_A_dn, lora_B_dn):
    out, _ = _run(hidden_states, gate_up_proj, down_proj,
                  lora_A_gu, lora_B_gu, lora_A_dn, lora_B_dn)
    return out


# revision 14
# speedup vs baseline: 1.1563x; 1.0468x over previous
"""Llama4 MoE experts + shared LoRA, expert-parallel on 8 TRN2 NeuronCores.

Per-core (expert e): x[1024,1024] @ W_gu[1024,4096] (+ rank-8 LoRA) -> SwiGLU
-> h[1024,2048] @ W_dn[2048,1024] (+ rank-8 LoRA) -> out[1024,1024].

Layout strategy: all contractions keep the reduced dim on partitions and the
intermediate transposed. x arrives from the host already transposed ([H, T]),
as do the tiny LoRA factors, so the kernel runs zero on-chip transposes:
gate_up^T tiles [128 F-part, 512 T-free] come out of PSUM with H contracted,
SwiGLU produces hiddenT with I on partitions -- the contraction layout the
down matmul needs -- and the down output lands as natural [T, H] rows.

All matmul operands are bf16 (1 cycle/row, same PE speed as float32r but half
the HBM traffic and SBUF footprint; accumulation stays fp32 in PSUM and the
output is written fp32). Weight DMAs are shaped for >=1KiB contiguous runs.
Only the two hardware DGE queues (sync, scalar) are used -- the gpsimd queue
is a software path that degrades the whole core.
"""
import sys

sys.path.insert(0, "/opt/trn_rl_repo")

import numpy as np
import ml_dtypes

import concourse.bacc as bacc
import concourse.bass as bass
import concourse.mybir as mybir
import concourse.tile as tile
from concourse.bass_utils import run_bass_kernel_spmd

E = 8           # experts == cores
T = 1024        # tokens per expert
H = 1024        # hidden
I = 2048        # intermediate
F2 = 2 * I      # gate+up
R = 8           # lora rank
SCALING = 2.0   # lora_alpha / rank
P = 128         # partitions
NFREE = 512     # moving free-dim per matmul (one PSUM bank of fp32)
KH = H // P     # 8 k-tiles over H
KI = I // P     # 16 k-tiles over I
NT = T // NFREE     # 2 T-chunks
FC = 512            # gate-column chunk per weight DMA
NFC = I // FC       # 4 weight-chunk iterations
JW = FC // P        # 4 f-subtiles per chunk
KO_DN = 4           # down-proj k-tiles packed per DMA

F32 = mybir.dt.float32
F32R = mybir.dt.float32r
BF16 = mybir.dt.bfloat16


def build_kernel():
    nc = bacc.Bacc("TRN2", target_bir_lowering=False, debug=False)

    xT_d = nc.dram_tensor("xT", [H, T], BF16, kind="ExternalInput")
    wgu_d = nc.dram_tensor("w_gu", [H, F2], BF16, kind="ExternalInput")
    wdn_d = nc.dram_tensor("w_dn", [I, H], BF16, kind="ExternalInput")
    aguT_d = nc.dram_tensor("a_guT", [H, R], BF16, kind="ExternalInput")
    bguT_d = nc.dram_tensor("b_guT", [R, F2], F32R, kind="ExternalInput")
    adnT_d = nc.dram_tensor("a_dnT", [I, R], BF16, kind="ExternalInput")
    bdnT_d = nc.dram_tensor("b_dnT", [R, H], F32R, kind="ExternalInput")
    out_d = nc.dram_tensor("out", [T, H], F32, kind="ExternalOutput")

    with tile.TileContext(nc) as tc:
        with (
            tc.tile_pool(name="xT", bufs=1) as xT_pool,
            tc.tile_pool(name="hT", bufs=1) as hT_pool,
            tc.tile_pool(name="smalls", bufs=1) as small_pool,
            tc.tile_pool(name="wg", bufs=2) as wg_pool,
            tc.tile_pool(name="wu", bufs=2) as wu_pool,
            tc.tile_pool(name="wdn", bufs=3) as wdn_pool,
            tc.tile_pool(name="silu", bufs=3) as silu_pool,
            tc.tile_pool(name="outs", bufs=6) as out_pool,
            tc.tile_pool(name="ps", bufs=8, space="PSUM") as ps_pool,
        ):
            # ---- resident inputs, all pre-transposed on the host ----
            # DMA issue order is the latency-critical path at startup:
            # x t-half 0 + A_gu + wg/wu chunk 0 unblock the first compute.
            xt = xT_pool.tile([P, KH, T], BF16, tag="xT")
            nc.sync.dma_start(
                xt[:, :, 0:NFREE],
                xT_d[:, 0:NFREE].rearrange("(ko ki) t -> ki ko t", ki=P))
            aguT = small_pool.tile([P, KH, R], BF16, tag="aguT")
            nc.sync.dma_start(aguT[:],
                              aguT_d[:].rearrange("(ko ki) r -> ki ko r", ki=P))

            wgs, wus = [], []
            def load_wgu(fc):
                wg = wg_pool.tile([P, KH, FC], BF16, tag="wg")
                wu = wu_pool.tile([P, KH, FC], BF16, tag="wu")
                fg, fu = FC * fc, I + FC * fc
                nc.sync.dma_start(
                    wg[:], wgu_d[:, fg:fg + FC].rearrange(
                        "(ko ki) f -> ki ko f", ki=P))
                nc.sync.dma_start(
                    wu[:], wgu_d[:, fu:fu + FC].rearrange(
                        "(ko ki) f -> ki ko f", ki=P))
                wgs.append(wg)
                wus.append(wu)

            load_wgu(0)
            bguT = small_pool.tile([96 + R, F2], F32R, tag="bguT")
            for c in range(4):
                nc.sync.dma_start(bguT[32 * c:32 * c + R, :], bguT_d[:])
            nc.sync.dma_start(
                xt[:, :, NFREE:T],
                xT_d[:, NFREE:T].rearrange("(ko ki) t -> ki ko t", ki=P))
            adnT = small_pool.tile([P, KI, R], BF16, tag="adnT")
            nc.sync.dma_start(adnT[:],
                              adnT_d[:].rearrange("(ko ki) r -> ki ko r", ki=P))
            bdnT = small_pool.tile([96 + R, H], F32R, tag="bdnT")
            for c in range(4):
                nc.sync.dma_start(bdnT[32 * c:32 * c + R, :], bdnT_d[:])

            # ---- r1T = SCALING * (A_gu @ x^T): [8 R, 1024 T] ----
            r1T = small_pool.tile([96 + R, T], F32R, tag="r1T")
            def r1_chunk(t):
                ts = slice(NFREE * t, NFREE * (t + 1))
                ps = ps_pool.tile([R, NFREE], F32, tag="ps")
                for k in range(KH):
                    nc.tensor.matmul(ps[:], aguT[:, k, :], xt[:, k, ts],
                                     start=(k == 0), stop=(k == KH - 1))
                nc.vector.tensor_scalar_mul(r1T[0:R, ts], ps[:], SCALING)
                for c in range(1, 4):
                    nc.sync.dma_start(r1T[32 * c:32 * c + R, ts], r1T[0:R, ts])

            # ---- gate_up^T + SwiGLU -> hiddenT[i] [128 I, 1024 T] ----
            hT = [hT_pool.tile([P, T], BF16, tag=f"hT{k}", name=f"hT{k}")
                  for k in range(KI)]

            def gu_pair(fc, t, jh, interleave=True):
                # two j-subtiles as 4 interleaved chains -- consecutive group
                # starts/stops pipeline instead of each paying the boundary
                # latency. interleave=False keeps the g chains first so the
                # startup chunk runs before its wu DMA lands.
                fg, fu = FC * fc, I + FC * fc
                ts = slice(NFREE * t, NFREE * (t + 1))
                wg, wu = wgs[fc], wus[fc]
                js = (2 * jh, 2 * jh + 1)
                ps = [ps_pool.tile([P, NFREE], F32, tag="ps",
                                   name=f"gu{fc}_{t}_{jh}_{c}")
                      for c in range(4)]
                order = range(KH)
                if interleave:
                    for k in order:
                        for c, j in enumerate(js):
                            fs = slice(P * j, P * (j + 1))
                            nc.tensor.matmul(ps[2 * c][:], wg[:, k, fs],
                                             xt[:, k, ts], start=(k == 0),
                                             stop=False)
                            nc.tensor.matmul(ps[2 * c + 1][:], wu[:, k, fs],
                                             xt[:, k, ts], start=(k == 0),
                                             stop=False)
                else:
                    for c, j in enumerate(js):
                        fs = slice(P * j, P * (j + 1))
                        for k in order:
                            nc.tensor.matmul(ps[2 * c][:], wg[:, k, fs],
                                             xt[:, k, ts], start=(k == 0),
                                             stop=False)
                    for c, j in enumerate(js):
                        fs = slice(P * j, P * (j + 1))
                        for k in order:
                            nc.tensor.matmul(ps[2 * c + 1][:], wu[:, k, fs],
                                             xt[:, k, ts], start=(k == 0),
                                             stop=False)
                for c, j in enumerate(js):
                    pg, pu = 64 * c, 64 * c + 32
                    nc.tensor.matmul(ps[2 * c][:],
                                     bguT[pg:pg + R, fg + P * j:fg + P * (j + 1)],
                                     r1T[pg:pg + R, ts], start=False, stop=True,
                                     tile_position=(pg, 0))
                    nc.tensor.matmul(ps[2 * c + 1][:],
                                     bguT[pu:pu + R, fu + P * j:fu + P * (j + 1)],
                                     r1T[pu:pu + R, ts], start=False, stop=True,
                                     tile_position=(pu, 0))
                for c, j in enumerate(js):
                    i = JW * fc + j
                    sg = silu_pool.tile([P, NFREE], F32, tag="silu")
                    nc.scalar.activation(sg[:], ps[2 * c][:],
                                         mybir.ActivationFunctionType.Silu)
                    nc.vector.tensor_mul(hT[i][:, ts], sg[:], ps[2 * c + 1][:])

            # fc0 runs t-major so compute starts before x t-half 1 lands
            r1_chunk(0)
            gu_pair(0, 0, 0, interleave=False)
            gu_pair(0, 0, 1)
            r1_chunk(1)
            gu_pair(0, 1, 0)
            gu_pair(0, 1, 1)
            for fc in range(1, NFC):
                load_wgu(fc)
                for t in range(NT):
                    for jh in range(JW // 2):
                        gu_pair(fc, t, jh)

            # ---- r2T = SCALING * (A_dn @ hidden^T): [8 R, 1024 T] ----
            r2T = small_pool.tile([96 + R, T], F32R, tag="r2T")
            for t in range(NT):
                ts = slice(NFREE * t, NFREE * (t + 1))
                ps = ps_pool.tile([R, NFREE], F32, tag="ps")
                for k in range(KI):
                    nc.tensor.matmul(ps[:], adnT[:, k, :], hT[k][:, ts],
                                     start=(k == 0), stop=(k == KI - 1))
                nc.vector.tensor_scalar_mul(r2T[0:R, ts], ps[:], SCALING)
                for c in range(1, 4):
                    nc.sync.dma_start(r2T[32 * c:32 * c + R, ts], r2T[0:R, ts])

            # ---- out[T, H] = hidden @ W_dn + lora ----
            for hg in range(NT):
                hs = slice(NFREE * hg, NFREE * (hg + 1))
                pos = [ps_pool.tile([P, NFREE], F32, tag="ps",
                                    name=f"po{hg}_{j}") for j in range(T // P)]
                for kg in range(KI // KO_DN):
                    wd = wdn_pool.tile([P, KO_DN, NFREE], BF16, tag="wdn")
                    r0 = P * KO_DN * kg
                    nc.sync.dma_start(
                        wd[:], wdn_d[r0:r0 + P * KO_DN, hs].rearrange(
                            "(ko ki) h -> ki ko h", ki=P))
                    for ko in range(KO_DN):
                        k = KO_DN * kg + ko
                        for j in range(T // P):
                            nc.tensor.matmul(pos[j][:],
                                             hT[k][:, P * j:P * (j + 1)],
                                             wd[:, ko, :],
                                             start=(k == 0), stop=False)
                for j in range(T // P):
                    pc = 32 * (j % 4)
                    nc.tensor.matmul(pos[j][:],
                                     r2T[pc:pc + R, P * j:P * (j + 1)],
                                     bdnT[pc:pc + R, hs], start=False, stop=True,
                                     tile_position=(pc, 0))
                    ot = out_pool.tile([P, NFREE], F32, tag="outs")
                    # alternate copy engines and DMA queues so the final
                    # drain runs two-wide
                    if j % 2 == 0:
                        nc.scalar.activation(ot[:], pos[j][:],
                                             mybir.ActivationFunctionType.Copy)
                        nc.sync.dma_start(out_d[P * j:P * (j + 1), hs], ot[:])
                    else:
                        nc.vector.tensor_copy(ot[:], pos[j][:])
                        nc.scalar.dma_start(out_d[P * j:P * (j + 1), hs], ot[:])

    nc.finalize()
    return nc


_NC_CACHE = None


def _get_nc():
    global _NC_CACHE
    if _NC_CACHE is None:
        _NC_CACHE = build_kernel()
    return _NC_CACHE


def _run(hidden_states, gate_up_proj, down_proj,
         lora_A_gu, lora_B_gu, lora_A_dn, lora_B_dn, **spmd_kwargs):
    bf16 = ml_dtypes.bfloat16
    xT = np.asarray(hidden_states).T.astype(bf16)      # [H, E*T] view->cast
    wgu = np.asarray(gate_up_proj).astype(bf16)
    wdn = np.asarray(down_proj).astype(bf16)
    aguT = np.ascontiguousarray(np.asarray(lora_A_gu).T.astype(bf16))
    bguT = np.ascontiguousarray(np.asarray(lora_B_gu, dtype=np.float32).T)
    adnT = np.ascontiguousarray(np.asarray(lora_A_dn).T.astype(bf16))
    bdnT = np.ascontiguousarray(np.asarray(lora_B_dn, dtype=np.float32).T)

    nc = _get_nc()
    in_maps = []
    for e in range(E):
        in_maps.append({
            "xT": np.ascontiguousarray(xT[:, T * e:T * (e + 1)]),
            "w_gu": wgu[e],
            "w_dn": wdn[e],
            "a_guT": aguT,
            "b_guT": bguT,
            "a_dnT": adnT,
            "b_dnT": bdnT,
        })
    res = run_bass_kernel_spmd(nc, in_maps, core_ids=list(range(E)),
                               **spmd_kwargs)
    out = np.concatenate([res.results[e]["out"] for e in range(E)], axis=0)
    return out.astype(np.float32), res


def kernel(hidden_states, gate_up_proj, down_proj,
           lora_A_gu, lora_B_gu, lora_A_dn, lora_B_dn):
    out, _ = _run(hidden_states, gate_up_proj, down_proj,
                  lora_A_gu, lora_B_gu, lora_A_dn, lora_B_dn)
    return out


# revision 18
# speedup vs baseline: 1.1873x; 1.0268x over previous
"""Llama4 MoE experts + shared LoRA, expert-parallel on 8 TRN2 NeuronCores.

Per-core (expert e): x[1024,1024] @ W_gu[1024,4096] (+ rank-8 LoRA) -> SwiGLU
-> h[1024,2048] @ W_dn[2048,1024] (+ rank-8 LoRA) -> out[1024,1024].

Layout strategy: all contractions keep the reduced dim on partitions and the
intermediate transposed. x arrives from the host already transposed ([H, T]),
as do the tiny LoRA factors, so the kernel runs zero on-chip transposes:
gate_up^T tiles [128 F-part, 512 T-free] come out of PSUM with H contracted,
SwiGLU produces hiddenT with I on partitions -- the contraction layout the
down matmul needs -- and the down output lands as natural [T, H] rows.

All matmul operands are bf16 (1 cycle/row, same PE speed as float32r but half
the HBM traffic and SBUF footprint; accumulation stays fp32 in PSUM and the
output is written fp32). Weight DMAs are shaped for >=1KiB contiguous runs.
Only the two hardware DGE queues (sync, scalar) are used -- the gpsimd queue
is a software path that degrades the whole core.
"""
import sys

sys.path.insert(0, "/opt/trn_rl_repo")

import numpy as np
import ml_dtypes

import concourse.bacc as bacc
import concourse.bass as bass
import concourse.mybir as mybir
import concourse.tile as tile
from concourse.bass_utils import run_bass_kernel_spmd

E = 8           # experts == cores
T = 1024        # tokens per expert
H = 1024        # hidden
I = 2048        # intermediate
F2 = 2 * I      # gate+up
R = 8           # lora rank
SCALING = 2.0   # lora_alpha / rank
P = 128         # partitions
NFREE = 512     # moving free-dim per matmul (one PSUM bank of fp32)
KH = H // P     # 8 k-tiles over H
KI = I // P     # 16 k-tiles over I
NT = T // NFREE     # 2 T-chunks
FC = 512            # gate-column chunk per weight DMA
NFC = I // FC       # 4 weight-chunk iterations
JW = FC // P        # 4 f-subtiles per chunk
KO_DN = 4           # down-proj k-tiles packed per DMA

F32 = mybir.dt.float32
F32R = mybir.dt.float32r
BF16 = mybir.dt.bfloat16


def build_kernel():
    nc = bacc.Bacc("TRN2", target_bir_lowering=False, debug=False)

    xT_d = nc.dram_tensor("xT", [H, T], BF16, kind="ExternalInput")
    wgu_d = nc.dram_tensor("w_gu", [H, F2], BF16, kind="ExternalInput")
    wdn_d = nc.dram_tensor("w_dn", [I, H], BF16, kind="ExternalInput")
    aguT_d = nc.dram_tensor("a_guT", [H, R], BF16, kind="ExternalInput")
    bguT_d = nc.dram_tensor("b_guT", [R, F2], F32R, kind="ExternalInput")
    adnT_d = nc.dram_tensor("a_dnT", [I, R], BF16, kind="ExternalInput")
    bdnT_d = nc.dram_tensor("b_dnT", [R, H], F32R, kind="ExternalInput")
    out_d = nc.dram_tensor("out", [T, H], F32, kind="ExternalOutput")

    with tile.TileContext(nc) as tc:
        with (
            tc.tile_pool(name="xT", bufs=1) as xT_pool,
            tc.tile_pool(name="hT", bufs=1) as hT_pool,
            tc.tile_pool(name="smalls", bufs=1) as small_pool,
            tc.tile_pool(name="wg", bufs=2) as wg_pool,
            tc.tile_pool(name="wu", bufs=2) as wu_pool,
            tc.tile_pool(name="wdn", bufs=6) as wdn_pool,
            tc.tile_pool(name="silu", bufs=3) as silu_pool,
            tc.tile_pool(name="outs", bufs=6) as out_pool,
            tc.tile_pool(name="ps", bufs=8, space="PSUM") as ps_pool,
        ):
            # ---- resident inputs, all pre-transposed on the host ----
            # DMA issue order is the latency-critical path at startup:
            # x t-half 0 + A_gu + wg/wu chunk 0 unblock the first compute.
            xt = xT_pool.tile([P, KH, T], BF16, tag="xT")
            nc.sync.dma_start(
                xt[:, :, 0:NFREE],
                xT_d[:, 0:NFREE].rearrange("(ko ki) t -> ki ko t", ki=P))
            aguT = small_pool.tile([P, KH, R], BF16, tag="aguT")
            nc.sync.dma_start(aguT[:],
                              aguT_d[:].rearrange("(ko ki) r -> ki ko r", ki=P))

            wgs, wus = [], []
            def load_wgu(fc):
                wg = wg_pool.tile([P, KH, FC], BF16, tag="wg")
                wu = wu_pool.tile([P, KH, FC], BF16, tag="wu")
                fg, fu = FC * fc, I + FC * fc
                nc.sync.dma_start(
                    wg[:], wgu_d[:, fg:fg + FC].rearrange(
                        "(ko ki) f -> ki ko f", ki=P))
                nc.sync.dma_start(
                    wu[:], wgu_d[:, fu:fu + FC].rearrange(
                        "(ko ki) f -> ki ko f", ki=P))
                wgs.append(wg)
                wus.append(wu)

            load_wgu(0)
            bguT = small_pool.tile([96 + R, F2], F32R, tag="bguT")
            for c in range(4):
                nc.sync.dma_start(bguT[32 * c:32 * c + R, :], bguT_d[:])
            nc.sync.dma_start(
                xt[:, :, NFREE:T],
                xT_d[:, NFREE:T].rearrange("(ko ki) t -> ki ko t", ki=P))
            adnT = small_pool.tile([P, KI, R], BF16, tag="adnT")
            nc.sync.dma_start(adnT[:],
                              adnT_d[:].rearrange("(ko ki) r -> ki ko r", ki=P))
            bdnT = small_pool.tile([96 + R, H], F32R, tag="bdnT")
            for c in range(4):
                nc.sync.dma_start(bdnT[32 * c:32 * c + R, :], bdnT_d[:])

            # ---- r1T = SCALING * (A_gu @ x^T): [8 R, 1024 T] ----
            r1T = small_pool.tile([96 + R, T], F32R, tag="r1T")
            def r1_chunk(t):
                ts = slice(NFREE * t, NFREE * (t + 1))
                ps = ps_pool.tile([R, NFREE], F32, tag="ps")
                for k in range(KH):
                    nc.tensor.matmul(ps[:], aguT[:, k, :], xt[:, k, ts],
                                     start=(k == 0), stop=(k == KH - 1))
                nc.vector.tensor_scalar_mul(r1T[0:R, ts], ps[:], SCALING)
                for c in range(1, 4):
                    nc.scalar.dma_start(r1T[32 * c:32 * c + R, ts], r1T[0:R, ts])

            # ---- gate_up^T + SwiGLU -> hiddenT[i] [128 I, 1024 T] ----
            hT = [hT_pool.tile([P, T], BF16, tag=f"hT{k}", name=f"hT{k}")
                  for k in range(KI)]

            def gu_pair(fc, t, jh, interleave=True):
                # two j-subtiles as 4 interleaved chains -- consecutive group
                # starts/stops pipeline instead of each paying the boundary
                # latency. interleave=False keeps the g chains first so the
                # startup chunk runs before its wu DMA lands.
                fg, fu = FC * fc, I + FC * fc
                ts = slice(NFREE * t, NFREE * (t + 1))
                wg, wu = wgs[fc], wus[fc]
                js = (2 * jh, 2 * jh + 1)
                ps = [ps_pool.tile([P, NFREE], F32, tag="ps",
                                   name=f"gu{fc}_{t}_{jh}_{c}")
                      for c in range(4)]
                order = range(KH)
                if interleave:
                    for k in order:
                        for c, j in enumerate(js):
                            fs = slice(P * j, P * (j + 1))
                            nc.tensor.matmul(ps[2 * c][:], wg[:, k, fs],
                                             xt[:, k, ts], start=(k == 0),
                                             stop=False)
                            nc.tensor.matmul(ps[2 * c + 1][:], wu[:, k, fs],
                                             xt[:, k, ts], start=(k == 0),
                                             stop=False)
                else:
                    for c, j in enumerate(js):
                        fs = slice(P * j, P * (j + 1))
                        for k in order:
                            nc.tensor.matmul(ps[2 * c][:], wg[:, k, fs],
                                             xt[:, k, ts], start=(k == 0),
                                             stop=False)
                    for c, j in enumerate(js):
                        fs = slice(P * j, P * (j + 1))
                        for k in order:
                            nc.tensor.matmul(ps[2 * c + 1][:], wu[:, k, fs],
                                             xt[:, k, ts], start=(k == 0),
                                             stop=False)
                for c, j in enumerate(js):
                    pg, pu = 64 * c, 64 * c + 32
                    nc.tensor.matmul(ps[2 * c][:],
                                     bguT[pg:pg + R, fg + P * j:fg + P * (j + 1)],
                                     r1T[pg:pg + R, ts], start=False, stop=True,
                                     tile_position=(pg, 0))
                    nc.tensor.matmul(ps[2 * c + 1][:],
                                     bguT[pu:pu + R, fu + P * j:fu + P * (j + 1)],
                                     r1T[pu:pu + R, ts], start=False, stop=True,
                                     tile_position=(pu, 0))
                for c, j in enumerate(js):
                    i = JW * fc + j
                    sg = silu_pool.tile([P, NFREE], F32, tag="silu")
                    nc.scalar.activation(sg[:], ps[2 * c][:],
                                         mybir.ActivationFunctionType.Silu)
                    nc.vector.tensor_mul(hT[i][:, ts], sg[:], ps[2 * c + 1][:])

            # fc0 runs t-major so compute starts before x t-half 1 lands
            r1_chunk(0)
            gu_pair(0, 0, 0, interleave=False)
            gu_pair(0, 0, 1)
            r1_chunk(1)
            gu_pair(0, 1, 0)
            gu_pair(0, 1, 1)
            for fc in range(1, NFC):
                load_wgu(fc)
                for t in range(NT):
                    for jh in range(JW // 2):
                        gu_pair(fc, t, jh)

            # ---- r2T = SCALING * (A_dn @ hidden^T): [8 R, 1024 T] ----
            r2T = small_pool.tile([96 + R, T], F32R, tag="r2T")
            for t in range(NT):
                ts = slice(NFREE * t, NFREE * (t + 1))
                ps = ps_pool.tile([R, NFREE], F32, tag="ps")
                for k in range(KI):
                    nc.tensor.matmul(ps[:], adnT[:, k, :], hT[k][:, ts],
                                     start=(k == 0), stop=(k == KI - 1))
                nc.vector.tensor_scalar_mul(r2T[0:R, ts], ps[:], SCALING)
                for c in range(1, 4):
                    nc.scalar.dma_start(r2T[32 * c:32 * c + R, ts], r2T[0:R, ts])

            # ---- out[T, H] = hidden @ W_dn + lora ----
            # 4 staggered sections (hg x j-group of 4): outputs drain every
            # quarter instead of all at once at the end; wd tiles are shared
            # by both j-groups of an hg
            for hg in range(NT):
                hs = slice(NFREE * hg, NFREE * (hg + 1))
                wds = []
                for jg in range(2):
                    jlist = list(range(4 * jg, 4 * jg + 4))
                    pos = [ps_pool.tile([P, NFREE], F32, tag="ps",
                                        name=f"po{hg}_{j}") for j in jlist]
                    for kg in range(KI // KO_DN):
                        if jg == 0:
                            wd = wdn_pool.tile([P, KO_DN, NFREE], BF16,
                                               tag="wdn", name=f"wd{hg}_{kg}")
                            r0 = P * KO_DN * kg
                            nc.sync.dma_start(
                                wd[:], wdn_d[r0:r0 + P * KO_DN, hs].rearrange(
                                    "(ko ki) h -> ki ko h", ki=P))
                            wds.append(wd)
                        wd = wds[kg]
                        for ko in range(KO_DN):
                            k = KO_DN * kg + ko
                            for c, j in enumerate(jlist):
                                nc.tensor.matmul(pos[c][:],
                                                 hT[k][:, P * j:P * (j + 1)],
                                                 wd[:, ko, :],
                                                 start=(k == 0), stop=False)
                    for c, j in enumerate(jlist):
                        pc = 32 * c
                        nc.tensor.matmul(pos[c][:],
                                         r2T[pc:pc + R, P * j:P * (j + 1)],
                                         bdnT[pc:pc + R, hs], start=False,
                                         stop=True, tile_position=(pc, 0))
                    for c, j in enumerate(jlist):
                        ot = out_pool.tile([P, NFREE], F32, tag="outs")
                        if c % 2 == 0:
                            nc.scalar.activation(ot[:], pos[c][:],
                                                 mybir.ActivationFunctionType.Copy)
                        else:
                            nc.vector.tensor_copy(ot[:], pos[c][:])
                        if c % 2 == 0:
                            nc.sync.dma_start(out_d[P * j:P * (j + 1), hs], ot[:])
                        else:
                            nc.scalar.dma_start(out_d[P * j:P * (j + 1), hs], ot[:])

    nc.finalize()
    return nc


_NC_CACHE = None


def _get_nc():
    global _NC_CACHE
    if _NC_CACHE is None:
        _NC_CACHE = build_kernel()
    return _NC_CACHE


def _run(hidden_states, gate_up_proj, down_proj,
         lora_A_gu, lora_B_gu, lora_A_dn, lora_B_dn, **spmd_kwargs):
    bf16 = ml_dtypes.bfloat16
    xT = np.asarray(hidden_states).T.astype(bf16)      # [H, E*T] view->cast
    wgu = np.asarray(gate_up_proj).astype(bf16)
    wdn = np.asarray(down_proj).astype(bf16)
    aguT = np.ascontiguousarray(np.asarray(lora_A_gu).T.astype(bf16))
    bguT = np.ascontiguousarray(np.asarray(lora_B_gu, dtype=np.float32).T)
    adnT = np.ascontiguousarray(np.asarray(lora_A_dn).T.astype(bf16))
    bdnT = np.ascontiguousarray(np.asarray(lora_B_dn, dtype=np.float32).T)

    nc = _get_nc()
    in_maps = []
    for e in range(E):
        in_maps.append({
            "xT": np.ascontiguousarray(xT[:, T * e:T * (e + 1)]),
            "w_gu": wgu[e],
            "w_dn": wdn[e],
            "a_guT": aguT,
            "b_guT": bguT,
            "a_dnT": adnT,
            "b_dnT": bdnT,
        })
    res = run_bass_kernel_spmd(nc, in_maps, core_ids=list(range(E)),
                               **spmd_kwargs)
    out = np.concatenate([res.results[e]["out"] for e in range(E)], axis=0)
    return out.astype(np.float32), res


def kernel(hidden_states, gate_up_proj, down_proj,
           lora_A_gu, lora_B_gu, lora_A_dn, lora_B_dn):
    out, _ = _run(hidden_states, gate_up_proj, down_proj,
                  lora_A_gu, lora_B_gu, lora_A_dn, lora_B_dn)
    return out
